# revision 6
# baseline (speedup 1.0000x reference)
"""Trainium2 Bass kernel for nn_MultiHeadAttention_52871047414119.

Reference (B=4, T=2048, D=512, H=8, DH=64, causal, eval):
    qkv = x @ w_qkv; per-head q,k,v
    out = concat_h(softmax(causal(q k^T / 8)) v) @ w_out

Sharding: 8 cores = 4 batches x 2 head-groups (4 heads each). Each core
returns the partial out-projection for its head group; the host adds the
two partials per batch (kernel() handles slicing + reduction).

Per-core program (one fused Tile kernel, all matmuls float32r):
  - single PSUM pool: tag "s" (3 bufs x 2 banks) shared by the S tiles and
    every other accumulator (x^T transposes, QKV chains, out-proj); tag
    "ot" (2 bufs x 1 bank) for the O^T accumulators. The 3-deep shared
    ring gives the S -> Exp -> PV chain enough lookahead that PE never
    waits on ScalarE's Exp latency.
  - attention processed per (qc, hp, h2), one head-pair-half per slot:
    slot = [deferred work][S pair: 2 matmuls][Exp][causal mask][prev PV]
    PV is always emitted one slot late (carried across h2/qc boundaries).
  - x^T via PE transposes in f32r (1.5 cycles/row vs 2.0 for fp32)
  - per-j causal lo: S and PV matmuls skip the fully-masked left half of
    each 128-wide k tile; Exp reads a stale-but-finite wedge there which
    affine_select zeroes before PV.
  - transposes + QKV chains for chunk qc+1 are deferred closures popped
    into qc's attention slots (guaranteed drained within the window so
    program order matches data deps); DMA order (x tiles 0-3, wq, wk, wv,
    rest, wo) starts the pipeline ~6us earlier.
  - normalize: denominator row 64 (ones column of Vaug) -> reciprocal ->
    gpsimd partition_broadcast -> one DVE multiply; ScalarE does nothing
    but Exp. The final q-chunk normalizes in 128-column chunks
    interleaved with its projections to shorten the tail.
"""

import sys

for _p in ("/opt/trn_rl_repo",):
    if _p not in sys.path:
        sys.path.insert(0, _p)

import numpy as np

import concourse.bass as bass  # noqa: F401  (registers types)
import concourse.tile as tile
import concourse.mybir as mybir
from concourse import bacc
from concourse.masks import make_identity

F32 = mybir.dt.float32
F32R = mybir.dt.float32r
AF = mybir.ActivationFunctionType
ALU = mybir.AluOpType

B, T, D, H, DH = 4, 2048, 512, 8, 64
NCORES = 8
HPC = 4          # heads per core
NTT = T // 128   # 16 row tiles
NDC = D // 128   # 4 contraction chunks
NQC = T // 512   # 4 q chunks
SCALE = 1.0 / np.sqrt(DH).item()

FUSED_RECIP = True  # reciprocal straight from PSUM row 64 to SBUF row 0


def emit_core_program(nc):
    x = nc.dram_tensor("x", [T, D], F32R, kind="ExternalInput").ap()
    wq = nc.dram_tensor("wq", [D, 256], F32R, kind="ExternalInput").ap()
    wk = nc.dram_tensor("wk", [D, 256], F32R, kind="ExternalInput").ap()
    wv = nc.dram_tensor("wv", [D, 256], F32R, kind="ExternalInput").ap()
    wo = nc.dram_tensor("wo", [256, D], F32R, kind="ExternalInput").ap()
    y = nc.dram_tensor("y", [T, D], F32, kind="ExternalOutput").ap()

    x_t = x.rearrange("(tt p) d -> p tt d", p=128)      # [128,16,512]
    wq_t = wq.rearrange("(dc p) m -> p dc m", p=128)    # [128,4,256]
    wk_t = wk.rearrange("(dc p) m -> p dc m", p=128)
    wv_t = wv.rearrange("(dc p) m -> p dc m", p=128)
    wo_t = wo.rearrange("(hp h2 dh) n -> (h2 dh) hp n", hp=2, h2=2)  # [128,2,512]
    y_t = y.rearrange("(tt p) d -> p tt d", p=128)

    with tile.TileContext(nc) as tc:
        with (
            tc.tile_pool(name="const", bufs=1) as constp,
            tc.tile_pool(name="wpool", bufs=1) as wpool,
            tc.tile_pool(name="xpool", bufs=1) as xpool,
            tc.tile_pool(name="big", bufs=1) as big,
            tc.tile_pool(name="ptp", bufs=8) as ptp,
            tc.tile_pool(name="smallp", bufs=4) as smallp,
            tc.tile_pool(name="yp", bufs=4) as yp,
            tc.tile_pool(name="psA", bufs=3, space="PSUM") as psA,
        ):
            ident_f32 = constp.tile([128, 128], F32)
            make_identity(nc, ident_f32)
            ident = constp.tile([128, 128], F32R)
            nc.vector.tensor_copy(ident, ident_f32)

            xb = xpool.tile([128, NTT, 512], F32R)
            wq_sb = wpool.tile([128, NDC, 256], F32R)
            wk_sb = wpool.tile([128, NDC, 256], F32R)
            wv_sb = wpool.tile([128, NDC, 256], F32R)
            wo_sb = wpool.tile([128, 2, 512], F32R)

            # DMA order: first q-chunk's x tiles, then the projection
            # weights the first QKV chains need, then the rest.
            for tt in range(4):
                nc.sync.dma_start(out=xb[:, tt, :], in_=x_t[:, tt, :])
            nc.sync.dma_start(out=wq_sb, in_=wq_t)
            nc.sync.dma_start(out=wk_sb, in_=wk_t)
            nc.sync.dma_start(out=wv_sb, in_=wv_t)
            for tt in range(4, NTT):
                nc.sync.dma_start(out=xb[:, tt, :], in_=x_t[:, tt, :])
            nc.sync.dma_start(out=wo_sb, in_=wo_t)

            xT = big.tile([128, NDC, T], F32R)     # x^T, d on partitions
            QT = big.tile([128, 2, T], F32R)       # head-pair packed (dh of 2 heads)
            KT = big.tile([128, 2, T], F32R)
            Vaug = big.tile([128, NTT, HPC, DH + 1], F32R)  # V natural + ones col
            OT = big.tile([128, 2, T], F32R)       # O^T: [64*h2+dh, hp, t]

            nc.vector.memset(Vaug.bitcast(F32)[:, :, :, 64:65], 1.0)  # denom col

            deferred = []
            norm_done = {}   # head index -> True once its norm was emitted

            def tr_closures(tg):
                def fn(tg, dc):
                    tr = psA.tile([128, 512], F32R, tag="s", name=f"tr{tg}_{dc}")
                    for i in range(4):
                        tt = 4 * tg + i
                        nc.tensor.transpose(
                            tr[:, i * 128:(i + 1) * 128],
                            xb[:, tt, dc * 128:(dc + 1) * 128],
                            ident,
                        )
                    nc.vector.tensor_copy(xT[:, dc, tg * 512:(tg + 1) * 512], tr)
                return [lambda tg=tg, dc=dc: fn(tg, dc) for dc in range(NDC)]

            def qkv_closures(qc):
                out = []

                def qk(w_sb, dst, hp, qc=qc):
                    acc = psA.tile([128, 512], F32, tag="s", name=f"qkv{qc}_{hp}")
                    for dc in range(NDC):
                        nc.tensor.matmul(
                            acc,
                            w_sb[:, dc, hp * 128:(hp + 1) * 128],
                            xT[:, dc, qc * 512:(qc + 1) * 512],
                            start=(dc == 0),
                            stop=(dc == NDC - 1),
                        )
                    nc.vector.tensor_copy(dst[:, hp, qc * 512:(qc + 1) * 512], acc)

                def vchain(tt):
                    acc = psA.tile([128, 256], F32, tag="s", name=f"vacc{tt}")
                    for dc in range(NDC):
                        nc.tensor.matmul(
                            acc,
                            xT[:, dc, tt * 128:(tt + 1) * 128],
                            wv_sb[:, dc, :],
                            start=(dc == 0),
                            stop=(dc == NDC - 1),
                        )
                    nc.vector.tensor_copy(
                        Vaug[:, tt, :, 0:64],
                        acc.rearrange("p (h x) -> p h x", h=HPC),
                    )

                for w_sb, dst in ((wq_sb, QT), (wk_sb, KT)):
                    for hp in range(2):
                        out.append(lambda w=w_sb, d=dst, hp=hp: qk(w, d, hp))
                out.extend(lambda tt=tt: vchain(tt)
                           for tt in range(4 * qc, 4 * qc + 4))
                return out

            def emit_proj(tt):
                acc = psA.tile([128, 512], F32, tag="s", name=f"yacc{tt}")
                for hp in range(2):
                    nc.tensor.matmul(
                        acc,
                        OT[:, hp, tt * 128:(tt + 1) * 128],
                        wo_sb[:, hp, :],
                        start=(hp == 0),
                        stop=(hp == 1),
                    )
                ysb = yp.tile([128, 512], F32, tag="ysb", name=f"ysb{tt}")
                nc.vector.tensor_copy(ysb, acc)
                nc.sync.dma_start(out=y_t[:, tt, :], in_=ysb)

            def recip_bcast(ot, key):
                rc = smallp.tile([1, 512], F32, tag="rc", name=f"rc{key}")
                if FUSED_RECIP:
                    nc.vector.reciprocal(rc, ot[64:65, :])
                else:
                    nc.vector.tensor_copy(rc, ot[64:65, :])
                    nc.vector.reciprocal(rc, rc)
                bcs = smallp.tile([64, 512], F32, tag="bcs", name=f"bcs{key}")
                nc.gpsimd.partition_broadcast(bcs, rc, channels=64)
                return bcs

            def emit_norm(ot, qc, hp, h2, hidx, with_projs):
                bcs = recip_bcast(ot, f"{qc}_{hp}_{h2}")
                nc.vector.tensor_tensor(
                    out=OT[64 * h2:64 * h2 + 64, hp,
                           qc * 512:(qc + 1) * 512],
                    in0=ot[0:64, :],
                    in1=bcs,
                    op=ALU.mult,
                )
                norm_done[hidx] = True
                if with_projs:
                    deferred.extend(
                        (lambda tt=tt: emit_proj(tt))
                        for tt in range(4 * qc, 4 * qc + 4)
                    )

            prev_pv = None   # (closure, norm-closure-or-None)
            window_left = [0]

            def run_slot_prelude():
                want = max(1, -(-len(deferred) // max(1, window_left[0])))
                for _ in range(min(len(deferred), min(want, 4))):
                    deferred.pop(0)()
                window_left[0] -= 1

            def run_prev_pv():
                nonlocal prev_pv
                if prev_pv is not None:
                    fn, norm = prev_pv
                    fn()
                    if norm is not None:
                        deferred.append(norm)
                    prev_pv = None

            # qc 0 prep runs inline; later chunks are deferred into the
            # previous chunk's attention slots.
            for fn in tr_closures(0):
                fn()
            for fn in qkv_closures(0):
                fn()

            hidx = 0
            for qc in range(NQC):
                if qc + 1 < NQC:
                    deferred.extend(tr_closures(qc + 1))
                    deferred.extend(qkv_closures(qc + 1))
                window_left[0] = 8 * (qc + 1)
                kt_max = 4 * (qc + 1)
                for hp in range(2):
                    for h2 in range(2):
                        # the "ot" ring is 2 deep: before reusing a buffer,
                        # its previous tile's norm must have been emitted.
                        while hidx >= 2 and not norm_done.get(hidx - 2, False):
                            if deferred:
                                deferred.pop(0)()
                            else:
                                run_prev_pv()
                        h = 2 * hp + h2
                        hb = 64 * h2
                        ot = psA.tile([65, 512], F32, tag="ot", bufs=2,
                                      name=f"ot{qc}_{hp}_{h2}")
                        for ktp in range(kt_max // 2):
                            lo = 256 if ktp == kt_max // 2 - 1 else 0
                            run_slot_prelude()
                            s = psA.tile([128, 2, 512], F32, tag="s",
                                         name=f"s{qc}_{hp}_{h2}_{ktp}")
                            los = []
                            for j in range(2):
                                kt = 2 * ktp + j
                                off = kt * 128 - qc * 512
                                lo_j = max(lo, min(off, 512)) if off > 0 else lo
                                los.append(lo_j)
                                # S written from pair-level lo (not lo_j):
                                # Exp reads [lo:], and CoreSim rejects reads
                                # of never-written PSUM. PV still skips the
                                # masked [lo, lo_j) wedge.
                                nc.tensor.matmul(
                                    s[:, j, lo:],
                                    KT[hb:hb + 64, hp, kt * 128:(kt + 1) * 128],
                                    QT[hb:hb + 64, hp,
                                       qc * 512 + lo:(qc + 1) * 512],
                                    start=True,
                                    stop=True,
                                )
                            pt = ptp.tile([128, 2, 512], F32R, tag="pt",
                                          name=f"pt{qc}_{hp}_{h2}_{ktp}")
                            nc.scalar.activation(pt[:, :, lo:], s[:, :, lo:],
                                                 AF.Exp, scale=SCALE)
                            for j in range(2):
                                kt = 2 * ktp + j
                                off = kt * 128 - qc * 512
                                if off >= 0:
                                    w = min(off + 128, 512)
                                    nc.gpsimd.affine_select(
                                        out=pt[:, j, lo:w],
                                        in_=pt[:, j, lo:w],
                                        pattern=[[1, w - lo]],
                                        compare_op=ALU.is_ge,
                                        fill=0.0,
                                        base=lo - off,
                                        channel_multiplier=-1,
                                    )
                            run_prev_pv()

                            def pv(pt=pt, ktp=ktp, los=los, ot=ot, h=h,
                                   kt_max=kt_max):
                                for j in range(2):
                                    kt = 2 * ktp + j
                                    nc.tensor.matmul(
                                        ot[:, los[j]:],
                                        Vaug[:, kt, h, :],
                                        pt[:, j, los[j]:],
                                        start=(kt == 0),
                                        stop=(kt == kt_max - 1),
                                        skip_group_check=True,
                                    )

                            norm = None
                            if ktp == kt_max // 2 - 1 and \
                                    (qc, hp, h2) != (NQC - 1, 1, 1):
                                norm = (lambda ot=ot, qc=qc, hp=hp, h2=h2,
                                        hidx=hidx, wp=(hp == 1 and h2 == 1):
                                        emit_norm(ot, qc, hp, h2, hidx, wp))
                            prev_pv = (pv, norm)
                        last_ot = ot
                        hidx += 1

            # tail: drain deferred, flush the last PV, then normalize the
            # final q-chunk in 128-column chunks interleaved with its
            # projections.
            while deferred:
                deferred.pop(0)()
            run_prev_pv()
            qc, hp, h2 = NQC - 1, 1, 1
            bcs = recip_bcast(last_ot, "last")
            for tt in range(4 * qc, 4 * qc + 4):
                c0 = (tt - 4 * qc) * 128
                nc.vector.tensor_tensor(
                    out=OT[64 * h2:64 * h2 + 64, hp,
                           tt * 128:(tt + 1) * 128],
                    in0=last_ot[0:64, c0:c0 + 128],
                    in1=bcs[:, c0:c0 + 128],
                    op=ALU.mult,
                )
                emit_proj(tt)

    return nc


_NC_CACHE = None


def get_nc():
    global _NC_CACHE
    if _NC_CACHE is None:
        nc = bacc.Bacc("TRN2", target_bir_lowering=False, debug=False,
                       num_devices=NCORES)
        emit_core_program(nc)
        nc.compile()
        _NC_CACHE = nc
    return _NC_CACHE


def make_in_maps(x, w_qkv, w_out):
    x = np.ascontiguousarray(np.asarray(x, dtype=np.float32))
    w_qkv = np.ascontiguousarray(np.asarray(w_qkv, dtype=np.float32))
    w_out = np.ascontiguousarray(np.asarray(w_out, dtype=np.float32))
    in_maps = []
    for c in range(NCORES):
        b, g = c // 2, c % 2
        lo = 256 * g
        in_maps.append({
            "x": np.ascontiguousarray(x[b]),
            "wq": np.ascontiguousarray(w_qkv[:, lo:lo + 256]),
            "wk": np.ascontiguousarray(w_qkv[:, 512 + lo:512 + lo + 256]),
            "wv": np.ascontiguousarray(w_qkv[:, 1024 + lo:1024 + lo + 256]),
            "wo": np.ascontiguousarray(w_out[lo:lo + 256, :]),
        })
    return in_maps


def assemble_output(results):
    out = np.empty((B, T, D), dtype=np.float32)
    for b in range(B):
        out[b] = results[2 * b]["y"] + results[2 * b + 1]["y"]
    return out


def kernel(x, w_qkv, w_out):
    from concourse.bass_utils import run_bass_kernel_spmd

    nc = get_nc()
    in_maps = make_in_maps(x, w_qkv, w_out)
    res = run_bass_kernel_spmd(nc, in_maps, list(range(NCORES))).results
    return assemble_output(res)


# revision 21
# speedup vs baseline: 1.0478x; 1.0478x over previous
"""Trainium2 Bass kernel for nn_MultiHeadAttention_52871047414119.

Reference (B=4, T=2048, D=512, H=8, DH=64, causal, eval):
    qkv = x @ w_qkv; per-head q,k,v
    out = concat_h(softmax(causal(q k^T / 8)) v) @ w_out

Sharding: 8 cores = 4 batches x 2 head-groups (4 heads each). Each core
returns the partial out-projection for its head group; the host adds the
two partials per batch (kernel() handles slicing + reduction).

Per-core program (one fused Tile kernel, all matmuls float32r):
  - single PSUM pool: tag "s" (3 bufs x 2 banks) shared by the S tiles and
    every other accumulator (x^T transposes, QKV chains, out-proj); tag
    "ot" (2 bufs x 1 bank) for the O^T accumulators. The 3-deep shared
    ring gives the S -> Exp -> PV chain enough lookahead that PE never
    waits on ScalarE's Exp latency.
  - attention processed per (qc, hp, h2), one head-pair-half per slot:
    slot = [deferred work][S pair: 2 matmuls][Exp][causal mask][prev PV]
    PV is always emitted one slot late (carried across h2/qc boundaries).
  - x^T via PE transposes in f32r (1.5 cycles/row vs 2.0 for fp32)
  - per-j causal lo: S and PV matmuls skip the fully-masked left half of
    each 128-wide k tile; Exp reads a stale-but-finite wedge there which
    affine_select zeroes before PV.
  - transposes + QKV chains for chunk qc+1 are deferred closures popped
    into qc's attention slots (guaranteed drained within the window so
    program order matches data deps); DMA order (x tiles 0-3, wq, wk, wv,
    rest, wo) starts the pipeline ~6us earlier.
  - normalize: denominator row 64 (ones column of Vaug) -> reciprocal ->
    gpsimd partition_broadcast -> one DVE multiply; ScalarE does nothing
    but Exp. The final q-chunk normalizes in 128-column chunks
    interleaved with its projections to shorten the tail.
"""

import sys

for _p in ("/opt/trn_rl_repo",):
    if _p not in sys.path:
        sys.path.insert(0, _p)

import numpy as np

import concourse.bass as bass  # noqa: F401  (registers types)
import concourse.tile as tile
import concourse.mybir as mybir
from concourse import bacc
from concourse.masks import make_identity

F32 = mybir.dt.float32
F32R = mybir.dt.float32r
AF = mybir.ActivationFunctionType
ALU = mybir.AluOpType

B, T, D, H, DH = 4, 2048, 512, 8, 64
NCORES = 8
HPC = 4          # heads per core
NTT = T // 128   # 16 row tiles
NDC = D // 128   # 4 contraction chunks
NQC = T // 512   # 4 q chunks
SCALE = 1.0 / np.sqrt(DH).item()

FUSED_RECIP = True  # reciprocal straight from PSUM row 64 to SBUF row 0


def emit_core_program(nc):
    x = nc.dram_tensor("x", [T, D], F32R, kind="ExternalInput").ap()
    wq = nc.dram_tensor("wq", [D, 256], F32R, kind="ExternalInput").ap()
    wk = nc.dram_tensor("wk", [D, 256], F32R, kind="ExternalInput").ap()
    wv = nc.dram_tensor("wv", [D, 256], F32R, kind="ExternalInput").ap()
    wo = nc.dram_tensor("wo", [256, D], F32R, kind="ExternalInput").ap()
    y = nc.dram_tensor("y", [T, D], F32, kind="ExternalOutput").ap()

    x_t = x.rearrange("(tt p) d -> p tt d", p=128)      # [128,16,512]
    wq_t = wq.rearrange("(dc p) m -> p dc m", p=128)    # [128,4,256]
    wk_t = wk.rearrange("(dc p) m -> p dc m", p=128)
    wv_t = wv.rearrange("(dc p) m -> p dc m", p=128)
    wo_t = wo.rearrange("(hp h2 dh) n -> (h2 dh) hp n", hp=2, h2=2)  # [128,2,512]
    y_t = y.rearrange("(tt p) d -> p tt d", p=128)

    with tile.TileContext(nc) as tc:
        with (
            tc.tile_pool(name="const", bufs=1) as constp,
            tc.tile_pool(name="wpool", bufs=1) as wpool,
            tc.tile_pool(name="xpool", bufs=1) as xpool,
            tc.tile_pool(name="big", bufs=1) as big,
            tc.tile_pool(name="ptp", bufs=8) as ptp,
            tc.tile_pool(name="smallp", bufs=4) as smallp,
            tc.tile_pool(name="yp", bufs=4) as yp,
            tc.tile_pool(name="psA", bufs=3, space="PSUM") as psA,
        ):
            ident_f32 = constp.tile([128, 128], F32)
            make_identity(nc, ident_f32)
            ident = constp.tile([128, 128], F32R)
            nc.vector.tensor_copy(ident, ident_f32)

            xb = xpool.tile([128, NTT, 512], F32R)
            wq_sb = wpool.tile([128, NDC, 256], F32R)
            wk_sb = wpool.tile([128, NDC, 256], F32R)
            wv_sb = wpool.tile([128, NDC, 256], F32R)
            wo_sb = wpool.tile([128, 2, 512], F32R)

            # DMA order: first q-chunk's x tiles, then the projection
            # weights the first QKV chains need, then the rest.
            for tt in range(4):
                nc.sync.dma_start(out=xb[:, tt, :], in_=x_t[:, tt, :])
            nc.sync.dma_start(out=wq_sb, in_=wq_t)
            nc.sync.dma_start(out=wk_sb, in_=wk_t)
            nc.sync.dma_start(out=wv_sb, in_=wv_t)
            for tt in range(4, NTT):
                nc.sync.dma_start(out=xb[:, tt, :], in_=x_t[:, tt, :])
            nc.sync.dma_start(out=wo_sb, in_=wo_t)

            xT = big.tile([128, NDC, T], F32R)     # x^T, d on partitions
            QT = big.tile([128, 2, T], F32R)       # head-pair packed (dh of 2 heads)
            KT = big.tile([128, 2, T], F32R)
            Vaug = big.tile([128, NTT, HPC, DH + 1], F32R)  # V natural + ones col
            OT = big.tile([128, 2, T], F32R)       # O^T: [64*h2+dh, hp, t]

            nc.vector.memset(Vaug.bitcast(F32)[:, :, :, 64:65], 1.0)  # denom col

            deferred = []
            norm_done = {}   # head index -> True once its norm was emitted

            def tr_closures(tg):
                def fn(tg, dc):
                    tr = psA.tile([128, 512], F32R, tag="s", name=f"tr{tg}_{dc}")
                    for i in range(4):
                        tt = 4 * tg + i
                        nc.tensor.transpose(
                            tr[:, i * 128:(i + 1) * 128],
                            xb[:, tt, dc * 128:(dc + 1) * 128],
                            ident,
                        )
                    nc.vector.tensor_copy(xT[:, dc, tg * 512:(tg + 1) * 512], tr)
                return [lambda tg=tg, dc=dc: fn(tg, dc) for dc in range(NDC)]

            def qkv_closures(qc):
                out = []

                def qk(w_sb, dst, hp, qc=qc):
                    acc = psA.tile([128, 512], F32, tag="s", name=f"qkv{qc}_{hp}")
                    for dc in range(NDC):
                        nc.tensor.matmul(
                            acc,
                            w_sb[:, dc, hp * 128:(hp + 1) * 128],
                            xT[:, dc, qc * 512:(qc + 1) * 512],
                            start=(dc == 0),
                            stop=(dc == NDC - 1),
                        )
                    nc.vector.tensor_copy(dst[:, hp, qc * 512:(qc + 1) * 512], acc)

                def vchain(tt):
                    acc = psA.tile([128, 256], F32, tag="s", name=f"vacc{tt}")
                    for dc in range(NDC):
                        nc.tensor.matmul(
                            acc,
                            xT[:, dc, tt * 128:(tt + 1) * 128],
                            wv_sb[:, dc, :],
                            start=(dc == 0),
                            stop=(dc == NDC - 1),
                        )
                    nc.vector.tensor_copy(
                        Vaug[:, tt, :, 0:64],
                        acc.rearrange("p (h x) -> p h x", h=HPC),
                    )

                for w_sb, dst in ((wq_sb, QT), (wk_sb, KT)):
                    for hp in range(2):
                        out.append(lambda w=w_sb, d=dst, hp=hp: qk(w, d, hp))
                out.extend(lambda tt=tt: vchain(tt)
                           for tt in range(4 * qc, 4 * qc + 4))
                return out

            proj_acc = {}

            def emit_proj_half(tt, hp):
                if hp == 0:
                    proj_acc[tt] = psA.tile([128, 512], F32, tag="s",
                                            name=f"yacc{tt}")
                nc.tensor.matmul(
                    proj_acc[tt],
                    OT[:, hp, tt * 128:(tt + 1) * 128],
                    wo_sb[:, hp, :],
                    start=(hp == 0),
                    stop=(hp == 1),
                )
                if hp == 1:
                    acc = proj_acc.pop(tt)
                    ysb = yp.tile([128, 512], F32, tag="ysb", name=f"ysb{tt}")
                    nc.vector.tensor_copy(ysb, acc)
                    nc.sync.dma_start(out=y_t[:, tt, :], in_=ysb)

            def emit_proj(tt):
                emit_proj_half(tt, 0)
                emit_proj_half(tt, 1)

            def emit_proj_tail(tt):
                acc = psA.tile([128, 512], F32, tag="s", name=f"yacc{tt}")
                for hp in range(2):
                    nc.tensor.matmul(
                        acc,
                        OT[:, hp, tt * 128:(tt + 1) * 128],
                        wo_sb[:, hp, :],
                        start=(hp == 0),
                        stop=(hp == 1),
                    )
                ysb = yp.tile([128, 512], F32, tag="ysb", name=f"ysb{tt}")
                nc.scalar.copy(ysb, acc)
                nc.sync.dma_start(out=y_t[:, tt, :], in_=ysb)

            def tail_norm_chunk(ot, hp, h2, q0, c0, c1, key):
                bcs = recip_bcast(ot, c0, c1, key)
                for tt in range((q0 + c0) // 128, (q0 + c1) // 128):
                    cc = tt * 128 - q0
                    nc.vector.tensor_tensor(
                        out=OT[64 * h2:64 * h2 + 64, hp,
                               tt * 128:(tt + 1) * 128],
                        in0=ot[0:64, cc:cc + 128],
                        in1=bcs[:, cc:cc + 128],
                        op=ALU.mult,
                    )
                    emit_proj_tail(tt)

            def recip_bcast(ot, key):
                rc = smallp.tile([1, 512], F32, tag="rc", name=f"rc{key}")
                if FUSED_RECIP:
                    nc.vector.reciprocal(rc, ot[64:65, :])
                else:
                    nc.vector.tensor_copy(rc, ot[64:65, :])
                    nc.vector.reciprocal(rc, rc)
                bcs = smallp.tile([64, 512], F32, tag="bcs", name=f"bcs{key}")
                nc.gpsimd.partition_broadcast(bcs, rc, channels=64)
                return bcs

            def emit_norm(ot, qc, hp, h2, hidx, with_projs):
                bcs = recip_bcast(ot, f"{qc}_{hp}_{h2}")
                nc.vector.tensor_tensor(
                    out=OT[64 * h2:64 * h2 + 64, hp,
                           qc * 512:(qc + 1) * 512],
                    in0=ot[0:64, :],
                    in1=bcs,
                    op=ALU.mult,
                )
                norm_done[hidx] = True
                if with_projs:
                    proj_q.extend(
                        (lambda tt=tt: emit_proj(tt))
                        for tt in range(4 * qc, 4 * qc + 4)
                    )

            prev_pv = None   # (closure, norm-closure-or-None)
            window_left = [0]
            in_last_window = [False]
            projs_open = [False]
            norm_q = []      # eager: cheap on PE, ring-critical
            proj_q = []      # saved for the ACT-bound later windows
            forced_q = []    # must run at the next slot (ring safety)
            vdef_q = []      # V chains: 1 per slot from window start

            pace_acc = [0.0]

            def run_slot_prelude():
                while forced_q:
                    forced_q.pop(0)()
                while norm_q:
                    norm_q.pop(0)()
                # error-diffusion pacing: spread deferred work evenly over
                # the remaining slots of the window instead of front-loading
                wl = max(1, window_left[0])
                supply = len(deferred) + (len(proj_q) if in_last_window[0]
                                          else 0)
                pace_acc[0] += supply / wl
                pops = min(int(pace_acc[0]), 4, supply)
                pace_acc[0] -= pops
                for _ in range(pops):
                    if deferred:
                        deferred.pop(0)()
                    elif in_last_window[0] and proj_q:
                        proj_q.pop(0)()
                    else:
                        break
                window_left[0] -= 1

            def run_prev_pv():
                nonlocal prev_pv
                if prev_pv is not None:
                    fn, norm = prev_pv
                    fn()
                    if norm is not None:
                        norm_q.append(norm)
                    prev_pv = None

            # qc 0 prep runs inline; later chunks are deferred into the
            # previous chunk's attention slots.
            for fn in tr_closures(0):
                fn()
            for fn in qkv_closures(0):
                fn()

            hidx = 0
            for qc in range(NQC):
                if qc + 1 < NQC:
                    deferred.extend(tr_closures(qc + 1))
                    deferred.extend(qkv_closures(qc + 1))
                window_left[0] = 8 * (qc + 1)
                in_last_window[0] = qc == NQC - 1
                kt_max = 4 * (qc + 1)
                for hp in range(2):
                    for h2 in range(2):
                        # the "ot" ring is 2 deep: before reusing a buffer,
                        # its previous tile's norm must have been emitted.
                        while hidx >= 2 and not norm_done.get(hidx - 2, False):
                            if norm_q:
                                norm_q.pop(0)()
                            elif deferred:
                                deferred.pop(0)()
                            else:
                                run_prev_pv()
                        h = 2 * hp + h2
                        hb = 64 * h2
                        ot = psA.tile([65, 512], F32, tag="ot", bufs=2,
                                      name=f"ot{qc}_{hp}_{h2}")
                        for ktp in range(kt_max // 2):
                            lo = 256 if ktp == kt_max // 2 - 1 else 0
                            run_slot_prelude()
                            s = psA.tile([128, 2, 512], F32, tag="s",
                                         name=f"s{qc}_{hp}_{h2}_{ktp}")
                            los = []
                            for j in range(2):
                                kt = 2 * ktp + j
                                off = kt * 128 - qc * 512
                                lo_j = max(lo, min(off, 512)) if off > 0 else lo
                                los.append(lo_j)
                                # S written from pair-level lo (not lo_j):
                                # Exp reads [lo:], and CoreSim rejects reads
                                # of never-written PSUM. PV still skips the
                                # masked [lo, lo_j) wedge.
                                nc.tensor.matmul(
                                    s[:, j, lo:],
                                    KT[hb:hb + 64, hp, kt * 128:(kt + 1) * 128],
                                    QT[hb:hb + 64, hp,
                                       qc * 512 + lo:(qc + 1) * 512],
                                    start=True,
                                    stop=True,
                                )
                            pt = ptp.tile([128, 2, 512], F32R, tag="pt",
                                          name=f"pt{qc}_{hp}_{h2}_{ktp}")
                            nc.scalar.activation(pt[:, :, lo:], s[:, :, lo:],
                                                 AF.Exp, scale=SCALE)
                            for j in range(2):
                                kt = 2 * ktp + j
                                off = kt * 128 - qc * 512
                                if off >= 0:
                                    w = min(off + 128, 512)
                                    nc.gpsimd.affine_select(
                                        out=pt[:, j, lo:w],
                                        in_=pt[:, j, lo:w],
                                        pattern=[[1, w - lo]],
                                        compare_op=ALU.is_ge,
                                        fill=0.0,
                                        base=lo - off,
                                        channel_multiplier=-1,
                                    )
                            run_prev_pv()

                            def pv(pt=pt, ktp=ktp, los=los, ot=ot, h=h,
                                   kt_max=kt_max):
                                for j in range(2):
                                    kt = 2 * ktp + j
                                    nc.tensor.matmul(
                                        ot[:, los[j]:],
                                        Vaug[:, kt, h, :],
                                        pt[:, j, los[j]:],
                                        start=(kt == 0),
                                        stop=(kt == kt_max - 1),
                                        skip_group_check=True,
                                    )

                            norm = None
                            if ktp == kt_max // 2 - 1 and \
                                    (qc, hp, h2) != (NQC - 1, 1, 1):
                                norm = (lambda ot=ot, qc=qc, hp=hp, h2=h2,
                                        hidx=hidx, wp=(hp == 1 and h2 == 1):
                                        emit_norm(ot, qc, hp, h2, hidx, wp))
                            prev_pv = (pv, norm)
                        last_ot = ot
                        hidx += 1
                # prep for qc+1 must be fully emitted before qc+1's S
                # matmuls read QT/KT/Vaug (program order = data order)
                while deferred:
                    deferred.pop(0)()

            # tail: drain deferred, flush the last PV, then normalize the
            # final q-chunk in 128-column chunks interleaved with its
            # projections.
            while norm_q or deferred or proj_q:
                if norm_q:
                    norm_q.pop(0)()
                elif deferred:
                    deferred.pop(0)()
                else:
                    proj_q.pop(0)()
            run_prev_pv()
            qc, hp, h2 = NQC - 1, 1, 1
            bcs = recip_bcast(last_ot, "last")
            for tt in range(4 * qc, 4 * qc + 4):
                c0 = (tt - 4 * qc) * 128
                nc.vector.tensor_tensor(
                    out=OT[64 * h2:64 * h2 + 64, hp,
                           tt * 128:(tt + 1) * 128],
                    in0=last_ot[0:64, c0:c0 + 128],
                    in1=bcs[:, c0:c0 + 128],
                    op=ALU.mult,
                )
                emit_proj(tt)

    return nc


_NC_CACHE = None


def get_nc():
    global _NC_CACHE
    if _NC_CACHE is None:
        nc = bacc.Bacc("TRN2", target_bir_lowering=False, debug=False,
                       num_devices=NCORES)
        emit_core_program(nc)
        nc.compile()
        _NC_CACHE = nc
    return _NC_CACHE


def make_in_maps(x, w_qkv, w_out):
    x = np.ascontiguousarray(np.asarray(x, dtype=np.float32))
    w_qkv = np.ascontiguousarray(np.asarray(w_qkv, dtype=np.float32))
    w_out = np.ascontiguousarray(np.asarray(w_out, dtype=np.float32))
    in_maps = []
    for c in range(NCORES):
        b, g = c // 2, c % 2
        lo = 256 * g
        in_maps.append({
            "x": np.ascontiguousarray(x[b]),
            "wq": np.ascontiguousarray(w_qkv[:, lo:lo + 256]),
            "wk": np.ascontiguousarray(w_qkv[:, 512 + lo:512 + lo + 256]),
            "wv": np.ascontiguousarray(w_qkv[:, 1024 + lo:1024 + lo + 256]),
            "wo": np.ascontiguousarray(w_out[lo:lo + 256, :]),
        })
    return in_maps


def assemble_output(results):
    out = np.empty((B, T, D), dtype=np.float32)
    for b in range(B):
        out[b] = results[2 * b]["y"] + results[2 * b + 1]["y"]
    return out


def kernel(x, w_qkv, w_out):
    from concourse.bass_utils import run_bass_kernel_spmd

    nc = get_nc()
    in_maps = make_in_maps(x, w_qkv, w_out)
    res = run_bass_kernel_spmd(nc, in_maps, list(range(NCORES))).results
    return assemble_output(res)
